# revision 5
# baseline (speedup 1.0000x reference)
"""3x3 grayscale dilation (all-ones SE) = 3x3 max-filter, stride 1, zero pad.

Input (8, 3, 1024, 1024) f32 -> same-shape output. 24 images, 3 per core.

fp16 end-to-end (rel tol 2e-2 >> fp16 rounding ~5e-4): halves DMA traffic
(12.6 MB/core vs the ~358 GB/s/core HBM bus) and doubles DVE throughput
(2-byte packed operands hit 2x_1p; measured 0.54-0.56 ns/elem on HW).

Device layout per image: [128 partitions, 8 rows, 2 (even|odd cols), 512];
host pre-packs (cast + column deinterleave + partition-major) so every DMA is
a contiguous >=4 KiB-per-partition descriptor run. Horizontal pair trick is
unit-stride:
    hp   = max(E, O); hmE = max(O[j-1], hp[j]); hmO = max(hp[j], E[j+1])
    (borders max(hp, 0) as small DVE tensor_scalar ops)
hm lands in rows 1..8 of a 10-row tile whose rows 0/9 are partition-halo rows
(zero padding at image edges), then vertical is a pure pair trick:
    vp[q] = max(hmx[2q+1], hmx[2q+2]);  vm[2s] = max(hmx[2s], vp[s]);
    vm[2s+1] = max(vp[s], hmx[2s+3])

Measured facts this schedule is built on (baseline trace, 67.1us total):
  start 11.5 (8.2 first DMA byte: ~3us system + ~2.5us ACT activation-table
  load hoisted ahead of the load issues + issue/first-byte) + DVE 43.4us
  essentially dense + 3.6us store drain + 8.6us teardown barrier. DVE is the
  bottleneck and the v3 ISA allows fp16 two-tensor max ONLY on DVE (Pool's
  TensorTensor is add/sub/mult/pow-only; ACT has no two-tensor ops), so the
  wins here are schedule-level:
   - halo rows via partition-shifted SBUF->SBUF DMA instead of PE-matmul ->
     PSUM -> ACT copy: no ACT compute op at all => no activation-table load
     => load DMAs issue ~2.5us earlier (ACT's first instructions are the
     loads). Corner zero rows are Pool memsets during the dead bring-up.
   - DVE stream reordered (h0,vp0 | h1 | vm0 | vp1 | h2 | vm1 | vp2 | vm2)
     so each image's halo DMA latency hides under the next image's h-stage.
   - stores split across both HWDGE rings; the last image stores in
     quarters with the final two quarters on different rings.
"""

import sys

sys.path.insert(0, "/opt/trn_rl_repo")

import numpy as np

N_CORES = 8
IMGS_PER_CORE = 3
H = W = 1024
R = 8  # rows per partition
P = 128
J = 512  # half-row (even/odd) length
FREE = R * 2 * J  # 8192 elems per partition per image

IMG0_CHUNKS = [(0, 2), (2, 4), (4, 6), (6, 8)]
LATE_CHUNKS = [(0, 4), (4, 8)]

_COMPILED_NC = None


def _build_nc():
    import concourse.mybir as mybir
    import concourse.tile as tile
    from concourse import bacc

    f16 = mybir.dt.float16
    MAX = mybir.AluOpType.max

    nc = bacc.Bacc(None)
    x = nc.declare_dram_parameter("input", [IMGS_PER_CORE, P, FREE], f16, isOutput=False)
    y = nc.declare_dram_parameter("output", [IMGS_PER_CORE, P, FREE], f16, isOutput=True)

    ROW1 = 1 * 2 * J
    ROW8 = 8 * 2 * J
    ROW9 = 9 * 2 * J

    with tile.TileContext(nc) as tc:
        with (
            tc.tile_pool(name="io", bufs=4) as io,
            tc.tile_pool(name="tmp", bufs=6) as tmp,
            tc.tile_pool(name="hmp", bufs=3) as hmp,
        ):
            X = [None] * IMGS_PER_CORE
            hmx = [None] * IMGS_PER_CORE

            # All hm tiles up front so their corner pad rows (partition 0 row
            # 0, partition 127 row 9 -- the image top/bottom zero padding the
            # halo shift DMAs can't deliver) can be zeroed on the otherwise
            # idle Pool engine during the ~8us bring-up window.
            for i in range(IMGS_PER_CORE):
                hmx[i] = hmp.tile([P, 10 * 2 * J], f16, tag="hm", name=f"hmx{i}")
                # Pool ops can't address a lone high partition; zero the full
                # halo rows instead -- the shift DMAs later overwrite all but
                # the true corner (image top/bottom) pad rows.
                nc.gpsimd.memset(hmx[i][:, 0 : 2 * J], 0.0)
                nc.gpsimd.memset(hmx[i][:, ROW9 : ROW9 + 2 * J], 0.0)

            # ALL loads on the scalar (fast, ~330 GB/s) ring, in consumption
            # order; one queue delivers bytes exactly in program order.
            for i in range(IMGS_PER_CORE):
                X[i] = io.tile([P, FREE], f16, tag="io", name=f"X{i}")
                chunks = IMG0_CHUNKS if i == 0 else LATE_CHUNKS
                for r0, r1 in chunks:
                    c0, c1 = r0 * 2 * J, r1 * 2 * J
                    nc.scalar.dma_start(out=X[i][:, c0:c1], in_=x[i][:, c0:c1])

            def h_stage(i, chunks):
                X3 = X[i][:].rearrange("p (r e j) -> p r e j", e=2, j=J)
                hp = tmp.tile([P, R * J], f16, tag="tmp", name=f"hp{i}")
                hp3 = hp[:].rearrange("p (r j) -> p r j", j=J)
                h3 = hmx[i][:].rearrange("p (r e j) -> p r e j", e=2, j=J)
                hmf = hmx[i][:].rearrange("p (r c) -> p r c", c=2 * J)
                hpf = hp[:].rearrange("p (r c) -> p r c", c=J)
                for r0, r1 in chunks:
                    nc.vector.tensor_tensor(
                        out=hp3[:, r0:r1],
                        in0=X3[:, r0:r1, 0],
                        in1=X3[:, r0:r1, 1],
                        op=MAX,
                    )
                    nc.vector.tensor_tensor(
                        out=h3[:, r0 + 1 : r1 + 1, 0, 1:J],
                        in0=X3[:, r0:r1, 1, 0 : J - 1],
                        in1=hp3[:, r0:r1, 1:J],
                        op=MAX,
                    )
                    nc.vector.tensor_tensor(
                        out=h3[:, r0 + 1 : r1 + 1, 1, 0 : J - 1],
                        in0=hp3[:, r0:r1, 0 : J - 1],
                        in1=X3[:, r0:r1, 0, 1:J],
                        op=MAX,
                    )
                    # both zero-pad border columns (E col 0 at flat 0, O col
                    # J-1 at flat 2J-1) in one strided max-with-0 on the idle
                    # Pool engine
                    nc.gpsimd.tensor_scalar_max(
                        out=hmf[:, r0 + 1 : r1 + 1, 0 : 2 * J : 2 * J - 1],
                        in0=hpf[:, r0:r1, 0 : J : J - 1],
                        scalar1=0.0,
                    )

            def halo_stage(i):
                # hmx row 0 of partition p = hm row 8 of partition p-1;
                # hmx row 9 of partition p = hm row 1 of partition p+1.
                # Partition-shifted SBUF->SBUF DMAs on the scalar HWDGE ring
                # (its queue is free once the loads are issued). Corner pad
                # rows were pre-zeroed by Pool memsets at kernel start.
                hm = hmx[i]
                nc.scalar.dma_start(
                    out=hm[1:P, 0 : 2 * J],
                    in_=hm[0 : P - 1, ROW8 : ROW8 + 2 * J],
                )
                nc.scalar.dma_start(
                    out=hm[0 : P - 1, ROW9 : ROW9 + 2 * J],
                    in_=hm[1:P, ROW1 : ROW1 + 2 * J],
                )

            def vp_stage(i):
                h3 = hmx[i][:].rearrange("p (r e j) -> p r e j", e=2, j=J)
                vp = tmp.tile([P, 4 * 2 * J], f16, tag="tmp", name=f"vp{i}")
                vp3 = vp[:].rearrange("p (q e j) -> p q e j", e=2, j=J)
                nc.vector.tensor_tensor(
                    out=vp3[:], in0=h3[:, 1:9:2], in1=h3[:, 2:10:2], op=MAX
                )
                return vp

            def vm_stage(i, vp, last):
                h3 = hmx[i][:].rearrange("p (r e j) -> p r e j", e=2, j=J)
                vp3 = vp[:].rearrange("p (q e j) -> p q e j", e=2, j=J)
                vm = io.tile([P, FREE], f16, tag="io")
                vm3 = vm[:].rearrange("p (r e j) -> p r e j", e=2, j=J)
                halves = [(0, 8)] if not last else [(0, 4), (4, 8)]
                for g0, g1 in halves:
                    s0, s1 = g0 // 2, g1 // 2
                    nc.vector.tensor_tensor(
                        out=vm3[:, g0:g1:2],
                        in0=h3[:, g0:g1:2],
                        in1=vp3[:, s0:s1],
                        op=MAX,
                    )
                    nc.vector.tensor_tensor(
                        out=vm3[:, g0 + 1 : g1 : 2],
                        in0=vp3[:, s0:s1],
                        in1=h3[:, g0 + 3 : g1 + 2 : 2],
                        op=MAX,
                    )
                # stores: both rings; last image in quarters so the tail is
                # two 0.5 MB transfers on different rings.
                if not last:
                    for (g0, g1), eng in (((0, 4), "sync"), ((4, 8), "scalar")):
                        getattr(nc, eng).dma_start(
                            out=y[i][:, g0 * 2 * J : g1 * 2 * J],
                            in_=vm[:, g0 * 2 * J : g1 * 2 * J],
                        )
                else:
                    for (g0, g1), eng in (
                        ((0, 4), "sync"),
                        ((4, 6), "scalar"),
                        ((6, 8), "sync"),
                    ):
                        getattr(nc, eng).dma_start(
                            out=y[i][:, g0 * 2 * J : g1 * 2 * J],
                            in_=vm[:, g0 * 2 * J : g1 * 2 * J],
                        )

            # DVE stream order: h0,vp0 | h1 | vm0 | vp1 | h2 | vm1 | vp2 | vm2
            # -- every halo DMA gets a full h-stage (or more) of DVE work to
            # hide under before the vm that consumes it.
            h_stage(0, IMG0_CHUNKS)
            halo_stage(0)
            vp0 = vp_stage(0)
            h_stage(1, LATE_CHUNKS)
            halo_stage(1)
            vm_stage(0, vp0, last=False)
            vp1 = vp_stage(1)
            h_stage(2, LATE_CHUNKS)
            halo_stage(2)
            vm_stage(1, vp1, last=False)
            vp2 = vp_stage(2)
            vm_stage(2, vp2, last=True)

    nc.compile()
    return nc


def _get_nc():
    global _COMPILED_NC
    if _COMPILED_NC is None:
        _COMPILED_NC = _build_nc()
    return _COMPILED_NC


def _pack(input):
    # (n, H, W) f32 -> (n, P, FREE) f16, per-partition [row][even|odd]
    x16 = input.reshape(-1, P, R, J, 2).astype(np.float16)
    return np.ascontiguousarray(x16.transpose(0, 1, 2, 4, 3)).reshape(-1, P, FREE)


def _unpack(out):
    # (n, P, FREE) f16 -> (n, H, W) f32
    o = out.reshape(-1, P, R, 2, J).transpose(0, 1, 2, 4, 3)
    return np.ascontiguousarray(o).reshape(-1, H, W).astype(np.float32)


def _reference_fallback(input, se):
    se = np.asarray(se, dtype=np.float32)
    se_h, se_w = se.shape
    pad_h, pad_w = se_h // 2, se_w // 2
    se_m1 = (se - 1.0).reshape(-1)
    padded = np.pad(input, ((0, 0), (0, 0), (pad_h, pad_h), (pad_w, pad_w)))
    out = None
    Hh, Ww = input.shape[2], input.shape[3]
    for i in range(se_h * se_w):
        xs, ys = i // se_h, i % se_h
        mask = np.float32(1.0) if se_m1[i] >= 0 else np.float32(0.0)
        contrib = mask * padded[:, :, xs : xs + Hh, ys : ys + Ww] + se_m1[i]
        out = contrib if out is None else np.maximum(out, contrib)
    return out


def kernel(input, se):
    from concourse.bass_utils import run_bass_kernel_spmd

    input = np.ascontiguousarray(np.asarray(input, dtype=np.float32))
    se_np = np.asarray(se, dtype=np.float32)
    if se_np.shape != (3, 3) or not np.all(se_np == 1.0) or input.shape != (
        8,
        3,
        H,
        W,
    ):
        return _reference_fallback(input, se_np).astype(np.float32)

    nc = _get_nc()
    flat = _pack(input.reshape(N_CORES * IMGS_PER_CORE, H, W))
    in_maps = [
        {"input": flat[k * IMGS_PER_CORE : (k + 1) * IMGS_PER_CORE]}
        for k in range(N_CORES)
    ]
    last_err = None
    for _attempt in range(3):
        try:
            res = run_bass_kernel_spmd(nc, in_maps, list(range(N_CORES)))
            out = np.concatenate(
                [res.results[k]["output"] for k in range(N_CORES)], axis=0
            )
            return _unpack(out).reshape(8, 3, H, W)
        except Exception as e:  # transient NRT_EXEC_UNIT_UNRECOVERABLE etc.
            last_err = e
    raise last_err


# revision 6
# speedup vs baseline: 1.5649x; 1.5649x over previous
"""3x3 grayscale dilation (all-ones SE) = 3x3 max-filter, stride 1, zero pad.

Input (8, 3, 1024, 1024) f32 -> same-shape output. 24 images, 3 per core.

fp16 end-to-end (rel tol 2e-2 >> fp16 rounding ~5e-4): halves DMA traffic
(12.6 MB/core vs the ~358 GB/s/core HBM bus) and doubles DVE throughput
(2-byte packed operands hit 2x_1p; measured 0.54-0.56 ns/elem on HW).

Device layout per image: [128 partitions, 8 rows, 2 (even|odd cols), 512];
host pre-packs (cast + column deinterleave + partition-major) so every DMA is
a contiguous >=4 KiB-per-partition descriptor run. Horizontal pair trick is
unit-stride:
    hp   = max(E, O); hmE = max(O[j-1], hp[j]); hmO = max(hp[j], E[j+1])
    (borders max(hp, 0) as small Pool tensor_scalar ops)
hm lands in rows 1..8 of a 10-row tile whose rows 0/9 are partition-halo rows
(zero padding at image edges), then vertical is a pure pair trick:
    vp[q] = max(hmx[2q+1], hmx[2q+2]);  vm[2s] = max(hmx[2s], vp[s]);
    vm[2s+1] = max(vp[s], hmx[2s+3])
Halos via PE shift-matmul -> PSUM -> ACT copy (partition-shifted SBUF->SBUF
DMA was measured 3x slower: +-1-partition shifts shatter into per-partition
cross-port descriptors, dma_active 94us vs 34us).

Measured facts this schedule is built on (baseline trace, 67.1us total):
  start 11.5us (8.2 first DMA byte -- runtime bring-up, queue-independent --
  plus ~2us HBM->SBUF completion-receipt latency on the first load sem) +
  DVE 43.4us essentially dense + 3.6us store drain + 8.6us teardown barrier.
  DVE is the bottleneck and the v3 ISA allows fp16 two-tensor max ONLY on
  DVE (Pool TensorTensor is add/sub/mult/pow-only; ACT has no two-tensor
  ops), so the deltas vs that baseline are schedule-level:
   - DVE stream reordered (h0,vp0 | h1 | vm0 | vp1 | h2 | vm1 | vp2 | vm2):
     each image's PE->PSUM->ACT halo chain gets a whole h-stage of DVE work
     to hide under instead of landing just-in-time.
   - vm emitted as one strided op pair per half-image group (img0/1 single
     (0,8) group): 8 fewer DVE instructions (~140ns fixed cost each).
   - stores split across both HWDGE rings with the last image in quarters,
     final two quarters on different rings so the tail drains in parallel.
"""

import sys

sys.path.insert(0, "/opt/trn_rl_repo")

import numpy as np

N_CORES = 8
IMGS_PER_CORE = 3
H = W = 1024
R = 8  # rows per partition
P = 128
J = 512  # half-row (even/odd) length
FREE = R * 2 * J  # 8192 elems per partition per image

IMG0_CHUNKS = [(0, 2), (2, 4), (4, 6), (6, 8)]
LATE_CHUNKS = [(0, 8)]

_COMPILED_NC = None


def _build_nc():
    import concourse.mybir as mybir
    import concourse.tile as tile
    from concourse import bacc

    f16 = mybir.dt.float16
    f32 = mybir.dt.float32
    MAX = mybir.AluOpType.max

    nc = bacc.Bacc(None)
    x = nc.declare_dram_parameter("input", [IMGS_PER_CORE, P, FREE], f16, isOutput=False)
    y = nc.declare_dram_parameter("output", [IMGS_PER_CORE, P, FREE], f16, isOutput=True)

    ROW1 = 1 * 2 * J
    ROW8 = 8 * 2 * J
    ROW9 = 9 * 2 * J

    with tile.TileContext(nc) as tc:
        with (
            tc.tile_pool(name="io", bufs=4) as io,
            tc.tile_pool(name="tmp", bufs=6) as tmp,
            tc.tile_pool(name="hmp", bufs=3) as hmp,
            tc.tile_pool(name="shp", bufs=1) as shp,
            tc.tile_pool(name="psum", bufs=2, space="PSUM") as psp,
        ):
            # Shifted identities (fp16) on the otherwise-idle Pool engine.
            sdn = shp.tile([P, P], f16, tag="sdn")
            sup = shp.tile([P, P], f16, tag="sup")
            for t, base in ((sdn, 1), (sup, -1)):
                nc.gpsimd.memset(t[:], 0.0)
                nc.gpsimd.affine_select(
                    out=t[:],
                    in_=t[:],
                    compare_op=mybir.AluOpType.not_equal,
                    fill=1.0,
                    base=base,
                    pattern=[[-1, P]],
                    channel_multiplier=1,
                )

            X = [None] * IMGS_PER_CORE
            hmx = [None] * IMGS_PER_CORE

            # ALL loads on the scalar (fast, ~330 GB/s) ring, in consumption
            # order. A second concurrent queue would starve image 0's small
            # chunks (DMA engines round-robin per DESCRIPTOR, so 16 KiB
            # descriptors on another queue get 4x the bytes of these 4 KiB
            # ones). One queue delivers bytes exactly in program order.
            for i in range(IMGS_PER_CORE):
                X[i] = io.tile([P, FREE], f16, tag="io", name=f"X{i}")
                if i == 0:
                    for r0, r1 in IMG0_CHUNKS:
                        c0, c1 = r0 * 2 * J, r1 * 2 * J
                        nc.scalar.dma_start(out=X[i][:, c0:c1], in_=x[i][:, c0:c1])
                else:
                    nc.scalar.dma_start(out=X[i][:], in_=x[i][:])

            def h_stage(i, chunks):
                X3 = X[i][:].rearrange("p (r e j) -> p r e j", e=2, j=J)
                hp = tmp.tile([P, R * J], f16, tag="tmp", name=f"hp{i}")
                hp3 = hp[:].rearrange("p (r j) -> p r j", j=J)
                hmx[i] = hmp.tile([P, 10 * 2 * J], f16, tag="hm", name=f"hmx{i}")
                h3 = hmx[i][:].rearrange("p (r e j) -> p r e j", e=2, j=J)
                hmf = hmx[i][:].rearrange("p (r c) -> p r c", c=2 * J)
                hpf = hp[:].rearrange("p (r c) -> p r c", c=J)
                for r0, r1 in chunks:
                    nc.vector.tensor_tensor(
                        out=hp3[:, r0:r1],
                        in0=X3[:, r0:r1, 0],
                        in1=X3[:, r0:r1, 1],
                        op=MAX,
                    )
                    nc.vector.tensor_tensor(
                        out=h3[:, r0 + 1 : r1 + 1, 0, 1:J],
                        in0=X3[:, r0:r1, 1, 0 : J - 1],
                        in1=hp3[:, r0:r1, 1:J],
                        op=MAX,
                    )
                    nc.vector.tensor_tensor(
                        out=h3[:, r0 + 1 : r1 + 1, 1, 0 : J - 1],
                        in0=hp3[:, r0:r1, 0 : J - 1],
                        in1=X3[:, r0:r1, 0, 1:J],
                        op=MAX,
                    )
                    # both zero-pad border columns (E col 0 at flat 0, O col
                    # J-1 at flat 2J-1) in one strided max-with-0 on the idle
                    # Pool engine (ACT would hoist a ~1.3us activation-table
                    # load ahead of the load DMA issues)
                    nc.gpsimd.tensor_scalar_max(
                        out=hmf[:, r0 + 1 : r1 + 1, 0 : 2 * J : 2 * J - 1],
                        in0=hpf[:, r0:r1, 0 : J : J - 1],
                        scalar1=0.0,
                    )

            def halo_stage(i):
                hm = hmx[i]
                ps = psp.tile([P, 2 * 2 * J], f32, tag="ps")  # [dh | uh]
                for c0 in (0, J):
                    nc.tensor.matmul(
                        ps[:, 2 * J + c0 : 2 * J + c0 + J],
                        sup[:],
                        hm[:, ROW1 + c0 : ROW1 + c0 + J],
                        start=True,
                        stop=True,
                    )
                for c0 in (0, J):
                    nc.tensor.matmul(
                        ps[:, c0 : c0 + J],
                        sdn[:],
                        hm[:, ROW8 + c0 : ROW8 + c0 + J],
                        start=True,
                        stop=True,
                    )
                nc.scalar.copy(out=hm[:, 0 : 2 * J], in_=ps[:, 0 : 2 * J])
                nc.scalar.copy(out=hm[:, ROW9:], in_=ps[:, 2 * J :])

            def vp_stage(i):
                h3 = hmx[i][:].rearrange("p (r e j) -> p r e j", e=2, j=J)
                vp = tmp.tile([P, 4 * 2 * J], f16, tag="tmp", name=f"vp{i}")
                vp3 = vp[:].rearrange("p (q e j) -> p q e j", e=2, j=J)
                nc.vector.tensor_tensor(
                    out=vp3[:], in0=h3[:, 1:9:2], in1=h3[:, 2:10:2], op=MAX
                )
                return vp

            def vm_stage(i, vp, last):
                h3 = hmx[i][:].rearrange("p (r e j) -> p r e j", e=2, j=J)
                vp3 = vp[:].rearrange("p (q e j) -> p q e j", e=2, j=J)
                vm = io.tile([P, FREE], f16, tag="io")
                vm3 = vm[:].rearrange("p (r e j) -> p r e j", e=2, j=J)
                groups = [(0, 8)] if not last else [(0, 4), (4, 8)]
                for g0, g1 in groups:
                    s0, s1 = g0 // 2, g1 // 2
                    nc.vector.tensor_tensor(
                        out=vm3[:, g0:g1:2],
                        in0=h3[:, g0:g1:2],
                        in1=vp3[:, s0:s1],
                        op=MAX,
                    )
                    nc.vector.tensor_tensor(
                        out=vm3[:, g0 + 1 : g1 : 2],
                        in0=vp3[:, s0:s1],
                        in1=h3[:, g0 + 3 : g1 + 2 : 2],
                        op=MAX,
                    )
                if not last:
                    for (g0, g1), eng in (((0, 4), "sync"), ((4, 8), "scalar" if i else "sync")):
                        getattr(nc, eng).dma_start(
                            out=y[i][:, g0 * 2 * J : g1 * 2 * J],
                            in_=vm[:, g0 * 2 * J : g1 * 2 * J],
                        )
                else:
                    for (g0, g1), eng in (
                        ((0, 4), "sync"),
                        ((4, 6), "scalar"),
                        ((6, 8), "sync"),
                    ):
                        getattr(nc, eng).dma_start(
                            out=y[i][:, g0 * 2 * J : g1 * 2 * J],
                            in_=vm[:, g0 * 2 * J : g1 * 2 * J],
                        )

            # DVE stream order: h0,vp0 | h1 | vm0 | vp1 | h2 | vm1 | vp2 | vm2
            # -- every halo chain gets a full h-stage of DVE work to hide
            # under before the vm that consumes it.
            h_stage(0, IMG0_CHUNKS)
            halo_stage(0)
            vp0 = vp_stage(0)
            h_stage(1, LATE_CHUNKS)
            halo_stage(1)
            vm_stage(0, vp0, last=False)
            vp1 = vp_stage(1)
            h_stage(2, LATE_CHUNKS)
            halo_stage(2)
            vm_stage(1, vp1, last=False)
            vp2 = vp_stage(2)
            vm_stage(2, vp2, last=True)

    nc.compile()
    return nc


def _get_nc():
    global _COMPILED_NC
    if _COMPILED_NC is None:
        _COMPILED_NC = _build_nc()
    return _COMPILED_NC


def _pack(input):
    # (n, H, W) f32 -> (n, P, FREE) f16, per-partition [row][even|odd]
    x16 = input.reshape(-1, P, R, J, 2).astype(np.float16)
    return np.ascontiguousarray(x16.transpose(0, 1, 2, 4, 3)).reshape(-1, P, FREE)


def _unpack(out):
    # (n, P, FREE) f16 -> (n, H, W) f32
    o = out.reshape(-1, P, R, 2, J).transpose(0, 1, 2, 4, 3)
    return np.ascontiguousarray(o).reshape(-1, H, W).astype(np.float32)


def _reference_fallback(input, se):
    se = np.asarray(se, dtype=np.float32)
    se_h, se_w = se.shape
    pad_h, pad_w = se_h // 2, se_w // 2
    se_m1 = (se - 1.0).reshape(-1)
    padded = np.pad(input, ((0, 0), (0, 0), (pad_h, pad_h), (pad_w, pad_w)))
    out = None
    Hh, Ww = input.shape[2], input.shape[3]
    for i in range(se_h * se_w):
        xs, ys = i // se_h, i % se_h
        mask = np.float32(1.0) if se_m1[i] >= 0 else np.float32(0.0)
        contrib = mask * padded[:, :, xs : xs + Hh, ys : ys + Ww] + se_m1[i]
        out = contrib if out is None else np.maximum(out, contrib)
    return out


def kernel(input, se):
    from concourse.bass_utils import run_bass_kernel_spmd

    input = np.ascontiguousarray(np.asarray(input, dtype=np.float32))
    se_np = np.asarray(se, dtype=np.float32)
    if se_np.shape != (3, 3) or not np.all(se_np == 1.0) or input.shape != (
        8,
        3,
        H,
        W,
    ):
        return _reference_fallback(input, se_np).astype(np.float32)

    nc = _get_nc()
    flat = _pack(input.reshape(N_CORES * IMGS_PER_CORE, H, W))
    in_maps = [
        {"input": flat[k * IMGS_PER_CORE : (k + 1) * IMGS_PER_CORE]}
        for k in range(N_CORES)
    ]
    last_err = None
    for _attempt in range(3):
        try:
            res = run_bass_kernel_spmd(nc, in_maps, list(range(N_CORES)))
            out = np.concatenate(
                [res.results[k]["output"] for k in range(N_CORES)], axis=0
            )
            return _unpack(out).reshape(8, 3, H, W)
        except Exception as e:  # transient NRT_EXEC_UNIT_UNRECOVERABLE etc.
            last_err = e
    raise last_err


# revision 9
# speedup vs baseline: 1.6387x; 1.0472x over previous
"""3x3 grayscale dilation (all-ones SE) = 3x3 max-filter, stride 1, zero pad.

Input (8, 3, 1024, 1024) f32 -> same-shape output. 24 images, 3 per core.

fp16 end-to-end (rel tol 2e-2 >> fp16 rounding ~5e-4): halves DMA traffic
(12.6 MB/core vs the ~358 GB/s/core HBM bus) and doubles DVE throughput
(2-byte packed operands hit 2x_1p; measured 0.54-0.56 ns/elem on HW).

Device layout per image: [128 partitions, 8 rows, 2 (even|odd cols), 512];
host pre-packs (cast + column deinterleave + partition-major) so every DMA is
a contiguous >=4 KiB-per-partition descriptor run. Horizontal pair trick is
unit-stride:
    hp   = max(E, O); hmE = max(O[j-1], hp[j]); hmO = max(hp[j], E[j+1])
    (borders max(hp, 0) as small Pool tensor_scalar ops)
hm lands in rows 1..8 of a 10-row tile whose rows 0/9 are partition-halo rows
(zero padding at image edges), then vertical is a pure pair trick:
    vp[q] = max(hmx[2q+1], hmx[2q+2]);  vm[2s] = max(hmx[2s], vp[s]);
    vm[2s+1] = max(vp[s], hmx[2s+3])
Halos via PE shift-matmul -> PSUM -> ACT copy (partition-shifted SBUF->SBUF
DMA was measured 3x slower: +-1-partition shifts shatter into per-partition
cross-port descriptors, dma_active 94us vs 34us).

Measured facts this schedule is built on (baseline trace, 67.1us total):
  start 11.5us (8.2 first DMA byte -- runtime bring-up, queue-independent --
  plus ~2us HBM->SBUF completion-receipt latency on the first load sem) +
  DVE 43.4us essentially dense + 3.6us store drain + 8.6us teardown barrier.
  DVE is the bottleneck and the v3 ISA allows fp16 two-tensor max ONLY on
  DVE (Pool TensorTensor is add/sub/mult/pow-only; ACT has no two-tensor
  ops), so the deltas vs that baseline are schedule-level:
   - DVE stream reordered (h0,vp0 | h1 | vm0 | vp1 | h2 | vm1 | vp2 | vm2):
     each image's PE->PSUM->ACT halo chain gets a whole h-stage of DVE work
     to hide under instead of landing just-in-time.
   - vm emitted as one strided op pair per half-image group (img0/1 single
     (0,8) group): 8 fewer DVE instructions (~140ns fixed cost each).
   - stores split across both HWDGE rings with the last image in quarters,
     final two quarters on different rings so the tail drains in parallel.
"""

import sys

sys.path.insert(0, "/opt/trn_rl_repo")

import numpy as np

N_CORES = 8
IMGS_PER_CORE = 3
H = W = 1024
R = 8  # rows per partition
P = 128
J = 512  # half-row (even/odd) length
FREE = R * 2 * J  # 8192 elems per partition per image

IMG0_CHUNKS = [(0, 2), (2, 4), (4, 6), (6, 8)]
LATE_CHUNKS = [(0, 8)]

_COMPILED_NC = None


def _build_nc():
    import concourse.mybir as mybir
    import concourse.tile as tile
    from concourse import bacc

    f16 = mybir.dt.float16
    f32 = mybir.dt.float32
    MAX = mybir.AluOpType.max

    nc = bacc.Bacc(None)
    x = nc.declare_dram_parameter("input", [IMGS_PER_CORE, P, FREE], f16, isOutput=False)
    y = nc.declare_dram_parameter("output", [IMGS_PER_CORE, P, FREE], f16, isOutput=True)

    ROW1 = 1 * 2 * J
    ROW8 = 8 * 2 * J
    ROW9 = 9 * 2 * J

    with tile.TileContext(nc) as tc:
        with (
            tc.tile_pool(name="io", bufs=4) as io,
            tc.tile_pool(name="tmp", bufs=6) as tmp,
            tc.tile_pool(name="hmp", bufs=3) as hmp,
            tc.tile_pool(name="shp", bufs=1) as shp,
            tc.tile_pool(name="psum", bufs=2, space="PSUM") as psp,
        ):
            # Shifted identities (fp16) on the otherwise-idle Pool engine.
            sdn = shp.tile([P, P], f16, tag="sdn")
            sup = shp.tile([P, P], f16, tag="sup")
            for t, base in ((sdn, 1), (sup, -1)):
                nc.gpsimd.memset(t[:], 0.0)
                nc.gpsimd.affine_select(
                    out=t[:],
                    in_=t[:],
                    compare_op=mybir.AluOpType.not_equal,
                    fill=1.0,
                    base=base,
                    pattern=[[-1, P]],
                    channel_multiplier=1,
                )

            X = [None] * IMGS_PER_CORE
            hmx = [None] * IMGS_PER_CORE

            # Loads: image 0's first two chunks ride the SYNC (SP) queue --
            # SP has no activation table, so it issues immediately after the
            # ~3us runtime bring-up, while ACT's issues sit behind the
            # ~2.5us act-table DMA its halo copies trigger. Everything else
            # stays on the scalar ring in consumption order. The sync-queue
            # chunks finish before the scalar ring reaches its 16 KiB
            # descriptors, so the per-descriptor round-robin between queues
            # never skews against them (both queues carry 4 KiB descriptors
            # while they overlap).
            for i in range(IMGS_PER_CORE):
                X[i] = io.tile([P, FREE], f16, tag="io", name=f"X{i}")
                if i == 0:
                    for ci, (r0, r1) in enumerate(IMG0_CHUNKS):
                        c0, c1 = r0 * 2 * J, r1 * 2 * J
                        eng = nc.sync if ci < 2 else nc.scalar
                        eng.dma_start(out=X[i][:, c0:c1], in_=x[i][:, c0:c1])
                else:
                    nc.scalar.dma_start(out=X[i][:], in_=x[i][:])

            def h_stage(i, chunks):
                X3 = X[i][:].rearrange("p (r e j) -> p r e j", e=2, j=J)
                hp = tmp.tile([P, R * J], f16, tag="tmp", name=f"hp{i}")
                hp3 = hp[:].rearrange("p (r j) -> p r j", j=J)
                hmx[i] = hmp.tile([P, 10 * 2 * J], f16, tag="hm", name=f"hmx{i}")
                h3 = hmx[i][:].rearrange("p (r e j) -> p r e j", e=2, j=J)
                hmf = hmx[i][:].rearrange("p (r c) -> p r c", c=2 * J)
                hpf = hp[:].rearrange("p (r c) -> p r c", c=J)
                for r0, r1 in chunks:
                    nc.vector.tensor_tensor(
                        out=hp3[:, r0:r1],
                        in0=X3[:, r0:r1, 0],
                        in1=X3[:, r0:r1, 1],
                        op=MAX,
                    )
                    nc.vector.tensor_tensor(
                        out=h3[:, r0 + 1 : r1 + 1, 0, 1:J],
                        in0=X3[:, r0:r1, 1, 0 : J - 1],
                        in1=hp3[:, r0:r1, 1:J],
                        op=MAX,
                    )
                    nc.vector.tensor_tensor(
                        out=h3[:, r0 + 1 : r1 + 1, 1, 0 : J - 1],
                        in0=hp3[:, r0:r1, 0 : J - 1],
                        in1=X3[:, r0:r1, 0, 1:J],
                        op=MAX,
                    )
                    # both zero-pad border columns (E col 0 at flat 0, O col
                    # J-1 at flat 2J-1) in one strided max-with-0 on the idle
                    # Pool engine (ACT would hoist a ~1.3us activation-table
                    # load ahead of the load DMA issues)
                    nc.gpsimd.tensor_scalar_max(
                        out=hmf[:, r0 + 1 : r1 + 1, 0 : 2 * J : 2 * J - 1],
                        in0=hpf[:, r0:r1, 0 : J : J - 1],
                        scalar1=0.0,
                    )

            def halo_stage(i):
                hm = hmx[i]
                ps = psp.tile([P, 2 * 2 * J], f32, tag="ps")  # [dh | uh]
                for c0 in (0, J):
                    nc.tensor.matmul(
                        ps[:, 2 * J + c0 : 2 * J + c0 + J],
                        sup[:],
                        hm[:, ROW1 + c0 : ROW1 + c0 + J],
                        start=True,
                        stop=True,
                    )
                for c0 in (0, J):
                    nc.tensor.matmul(
                        ps[:, c0 : c0 + J],
                        sdn[:],
                        hm[:, ROW8 + c0 : ROW8 + c0 + J],
                        start=True,
                        stop=True,
                    )
                nc.scalar.copy(out=hm[:, 0 : 2 * J], in_=ps[:, 0 : 2 * J])
                nc.scalar.copy(out=hm[:, ROW9:], in_=ps[:, 2 * J :])

            def vp_stage(i):
                h3 = hmx[i][:].rearrange("p (r e j) -> p r e j", e=2, j=J)
                vp = tmp.tile([P, 4 * 2 * J], f16, tag="tmp", name=f"vp{i}")
                vp3 = vp[:].rearrange("p (q e j) -> p q e j", e=2, j=J)
                nc.vector.tensor_tensor(
                    out=vp3[:], in0=h3[:, 1:9:2], in1=h3[:, 2:10:2], op=MAX
                )
                return vp

            def vm_stage(i, vp, last):
                h3 = hmx[i][:].rearrange("p (r e j) -> p r e j", e=2, j=J)
                vp3 = vp[:].rearrange("p (q e j) -> p q e j", e=2, j=J)
                vm = io.tile([P, FREE], f16, tag="io")
                vm3 = vm[:].rearrange("p (r e j) -> p r e j", e=2, j=J)
                groups = [(0, 8)] if not last else [(0, 4), (4, 8)]
                for g0, g1 in groups:
                    s0, s1 = g0 // 2, g1 // 2
                    nc.vector.tensor_tensor(
                        out=vm3[:, g0:g1:2],
                        in0=h3[:, g0:g1:2],
                        in1=vp3[:, s0:s1],
                        op=MAX,
                    )
                    nc.vector.tensor_tensor(
                        out=vm3[:, g0 + 1 : g1 : 2],
                        in0=vp3[:, s0:s1],
                        in1=h3[:, g0 + 3 : g1 + 2 : 2],
                        op=MAX,
                    )
                if not last:
                    rings = ("sync", "sync") if i == 0 else ("scalar", "scalar")
                    for (g0, g1), eng in zip(((0, 4), (4, 8)), rings):
                        getattr(nc, eng).dma_start(
                            out=y[i][:, g0 * 2 * J : g1 * 2 * J],
                            in_=vm[:, g0 * 2 * J : g1 * 2 * J],
                        )
                else:
                    # tail: last two quarters on different rings so they
                    # drain in parallel after the final DVE op
                    for (g0, g1), eng in (
                        ((0, 4), "sync"),
                        ((4, 6), "scalar"),
                        ((6, 8), "sync"),
                    ):
                        getattr(nc, eng).dma_start(
                            out=y[i][:, g0 * 2 * J : g1 * 2 * J],
                            in_=vm[:, g0 * 2 * J : g1 * 2 * J],
                        )

            # Baseline emission order: vm(i) directly after vp(i) so stores
            # start flowing as early as possible (delaying vm compresses the
            # store window into the tail -- measured +5us total).
            for i in range(IMGS_PER_CORE):
                h_stage(i, IMG0_CHUNKS if i == 0 else LATE_CHUNKS)
                halo_stage(i)
                vpi = vp_stage(i)
                vm_stage(i, vpi, last=(i == IMGS_PER_CORE - 1))

    nc.compile()
    return nc


def _get_nc():
    global _COMPILED_NC
    if _COMPILED_NC is None:
        _COMPILED_NC = _build_nc()
    return _COMPILED_NC


def _pack(input):
    # (n, H, W) f32 -> (n, P, FREE) f16, per-partition [row][even|odd]
    x16 = input.reshape(-1, P, R, J, 2).astype(np.float16)
    return np.ascontiguousarray(x16.transpose(0, 1, 2, 4, 3)).reshape(-1, P, FREE)


def _unpack(out):
    # (n, P, FREE) f16 -> (n, H, W) f32
    o = out.reshape(-1, P, R, 2, J).transpose(0, 1, 2, 4, 3)
    return np.ascontiguousarray(o).reshape(-1, H, W).astype(np.float32)


def _reference_fallback(input, se):
    se = np.asarray(se, dtype=np.float32)
    se_h, se_w = se.shape
    pad_h, pad_w = se_h // 2, se_w // 2
    se_m1 = (se - 1.0).reshape(-1)
    padded = np.pad(input, ((0, 0), (0, 0), (pad_h, pad_h), (pad_w, pad_w)))
    out = None
    Hh, Ww = input.shape[2], input.shape[3]
    for i in range(se_h * se_w):
        xs, ys = i // se_h, i % se_h
        mask = np.float32(1.0) if se_m1[i] >= 0 else np.float32(0.0)
        contrib = mask * padded[:, :, xs : xs + Hh, ys : ys + Ww] + se_m1[i]
        out = contrib if out is None else np.maximum(out, contrib)
    return out


def kernel(input, se):
    from concourse.bass_utils import run_bass_kernel_spmd

    input = np.ascontiguousarray(np.asarray(input, dtype=np.float32))
    se_np = np.asarray(se, dtype=np.float32)
    if se_np.shape != (3, 3) or not np.all(se_np == 1.0) or input.shape != (
        8,
        3,
        H,
        W,
    ):
        return _reference_fallback(input, se_np).astype(np.float32)

    nc = _get_nc()
    flat = _pack(input.reshape(N_CORES * IMGS_PER_CORE, H, W))
    in_maps = [
        {"input": flat[k * IMGS_PER_CORE : (k + 1) * IMGS_PER_CORE]}
        for k in range(N_CORES)
    ]
    last_err = None
    for _attempt in range(3):
        try:
            res = run_bass_kernel_spmd(nc, in_maps, list(range(N_CORES)))
            out = np.concatenate(
                [res.results[k]["output"] for k in range(N_CORES)], axis=0
            )
            return _unpack(out).reshape(8, 3, H, W)
        except Exception as e:  # transient NRT_EXEC_UNIT_UNRECOVERABLE etc.
            last_err = e
    raise last_err
